# revision 38
# baseline (speedup 1.0000x reference)
"""Causal self-attention (GPT-style, B=8 T=1024 C=768 H=12) on 8 Trainium2 cores.

Sharding: pure data parallel - core b computes batch element b end-to-end
(weights replicated per core). No collectives.

v4 design (all matmul operands bf16, PSUM f32):
  - PE warm-up burst at t=0 (HAM reaches K=8/8 before real matmuls)
  - per-pair W_attn column staging (attention starts ~15us in)
  - x cast to bf16 (ACT), PE-transpose -> xt
  - qkT per head-pair: W-stationary matmuls -> qt/kt [128,1024] bf16
    (head A on partitions 0:63, head B on 64:127)
  - vhat per t-chunk: x-stationary @ Wv + ones column per head (denominator
    via the ones-column trick)
  - ST row-tiled: heads A and B issued back-to-back at tile_position (0,0)
    and (64,0) -> concurrent in the PE array (2x)
  - exp on ACT (scale=1/8) -> pt bf16, triangular diag mask on GPSIMD
  - PV: vhat-stationary @ pt -> [65, q-half] PSUM, q processed in 512-halves
  - softmax scale: den row copy -> reciprocal (DVE) -> DMA stride-0
    partition broadcast -> DVE in-place multiply
  - proj split: pairs 0..3 accumulated into SBUF partials during pairs 4/5;
    final pass (ps_st pool, 4-deep) adds pairs 4,5 + partial, DMA out
"""

import sys

if "/opt/trn_rl_repo" not in sys.path:
    sys.path.insert(0, "/opt/trn_rl_repo")

import numpy as np

import concourse.bass as bass  # noqa: F401  (registers types)
import concourse.mybir as mybir
import concourse.tile as tile
from concourse import bacc
from concourse.masks import make_identity

F32 = mybir.dt.float32
BF16 = mybir.dt.bfloat16
AF = mybir.ActivationFunctionType

T = 1024
C = 768
H = 12
D = 64
TT = 8  # t tiles of 128
CC = 6  # c chunks of 128
PAIRS = 6  # head pairs
N3 = 3 * C


def build_nc():
    from contextlib import ExitStack

    nc = bacc.Bacc()
    x_d = nc.declare_dram_parameter("x", [T, C], F32, isOutput=False)
    wa_d = nc.declare_dram_parameter("wa", [C, N3], F32, isOutput=False)
    wp_d = nc.declare_dram_parameter("wp", [C, C], F32, isOutput=False)
    out_d = nc.declare_dram_parameter("out", [T, C], F32, isOutput=True)

    with tile.TileContext(nc) as tc:
        with ExitStack() as stack:
            pool = lambda name, bufs, **kw: stack.enter_context(  # noqa: E731
                tc.tile_pool(name=name, bufs=bufs, **kw)
            )
            singles = pool("singles", 1)
            xstage = pool("xstage", 3)
            xb_pool = pool("xb_pool", 1)
            xt_pool = pool("xt_pool", 1)
            wqs_pool = pool("wqs_pool", 4)
            wqk_pool = pool("wqk_pool", 18)
            wv_pool = pool("wv_pool", 1)
            wstage = pool("wstage", 2)
            wp_pool = pool("wp_pool", 1)
            qkt_pool = pool("qkt_pool", 3)
            vh_pool = pool("vh_pool", 1)
            pt_pool = pool("pt_pool", 6)
            yp_pool = pool("yp_pool", 1)
            rec_pool = pool("rec_pool", 2)
            bc_pool = pool("bc_pool", 2)
            part_pool = pool("part_pool", 1)
            outst_pool = pool("outst_pool", 2)
            dummy_pool = pool("dummy_pool", 1)
            ps_st = pool("ps_st", 4, space="PSUM")
            ps_pv = pool("ps_pv", 2, space="PSUM")
            ps_flex = pool("ps_flex", 2, space="PSUM")

            # ---- identity + PE warm-up bursts (HAM to K=8/8 during DMA) ----
            ident = singles.tile([128, 128], BF16)
            make_identity(nc, ident)
            wu_s = dummy_pool.tile([128, 32], F32, name="wu_s")

            def emit_warmup(n_mm):
                wu_ps = ps_st.tile([128, 512], F32, tag="st", name="wu")
                for k in range(n_mm):
                    nc.tensor.matmul(
                        wu_ps[:, 0:128], ident, ident, start=True, stop=True
                    )
                nc.vector.tensor_copy(out=wu_s, in_=wu_ps[:, 0:32])

            emit_warmup(80)

            # ---- DMA in: x, then per-pair qk cols, wv, wp ----
            xs = []
            for tt in range(TT):
                s = xstage.tile([128, C], F32, tag="xs", name="xs")
                nc.sync.dma_start(out=s, in_=x_d[tt * 128 : (tt + 1) * 128, :])
                xs.append(s)

            # absorb the ACT exp-table load while DMA-bound
            dummy = singles.tile([1, 32], F32)
            nc.vector.memset(dummy, 0.0)
            nc.scalar.activation(out=dummy, in_=dummy, func=AF.Exp, scale=0.125)

            wqk01 = {}  # (p, cc) -> [128, 256] bf16 (q cols | k cols), pairs 0/1

            def emit_wqk_dma(p):
                for cc in range(CC):
                    s = wqs_pool.tile([128, 256], F32, tag="wqs", name="wqs")
                    nc.sync.dma_start(
                        out=s[:, 0:128],
                        in_=wa_d[
                            cc * 128 : (cc + 1) * 128, 128 * p : 128 * (p + 1)
                        ],
                    )
                    nc.sync.dma_start(
                        out=s[:, 128:256],
                        in_=wa_d[
                            cc * 128 : (cc + 1) * 128,
                            C + 128 * p : C + 128 * (p + 1),
                        ],
                    )
                    b = wqk_pool.tile([128, 256], BF16, tag="wqk", name="wqk")
                    nc.scalar.copy(out=b, in_=s)
                    wqk01[(p, cc)] = b

            emit_wqk_dma(0)

            # wv DMA ahead of pair-1 weights: vhat is the big early PE block
            wv_bf = []
            for cc in range(CC):
                s = wstage.tile([128, C], F32, tag="wvs", name="wvs")
                nc.sync.dma_start(
                    out=s, in_=wa_d[cc * 128 : (cc + 1) * 128, 2 * C : 3 * C]
                )
                b = wv_pool.tile([128, C], BF16, name=f"wvbf{cc}")
                nc.scalar.copy(out=b, in_=s)
                wv_bf.append(b)

            emit_wqk_dma(1)

            def wq_slice(p, cc):
                if p < 2:
                    return wqk01[(p, cc)][:, 0:128]
                return wrest_bf[cc][:, 128 * p - 256 : 128 * p - 128]

            def wk_slice(p, cc):
                if p < 2:
                    return wqk01[(p, cc)][:, 128:256]
                return wrest_bf[cc][:, 256 + 128 * p : 256 + 128 * (p + 1)]

            # remaining q/k columns (pairs 2..5) as two efficient chunk DMAs
            wrest_bf = []
            for cc in range(CC):
                s = wstage.tile([128, 1024], F32, tag="wrs", name="wrs")
                nc.sync.dma_start(
                    out=s[:, 0:512],
                    in_=wa_d[cc * 128 : (cc + 1) * 128, 256:768],
                )
                nc.sync.dma_start(
                    out=s[:, 512:1024],
                    in_=wa_d[cc * 128 : (cc + 1) * 128, C + 256 : C + 768],
                )
                b = wqk_pool.tile([128, 1024], BF16, tag="wrb", name=f"wrb{cc}")
                nc.scalar.copy(out=b, in_=s)
                wrest_bf.append(b)

            # ---- cast x -> bf16 (ACT), PE-transpose -> xt ----
            xb = []
            for tt in range(TT):
                b = xb_pool.tile([128, C], BF16, name=f"xb{tt}")
                nc.scalar.copy(out=b, in_=xs[tt])
                xb.append(b)
            xt = []
            for cc in range(CC):
                t_ = xt_pool.tile([128, T], BF16, name=f"xt{cc}")
                xt.append(t_)
            for cc in range(CC):
                for tt4 in range(2):
                    trp = ps_flex.tile([128, 512], BF16, tag="flex", name="trp")
                    for k in range(4):
                        tt = 4 * tt4 + k
                        nc.tensor.transpose(
                            trp[:, 128 * k : 128 * (k + 1)],
                            xb[tt][:, cc * 128 : (cc + 1) * 128],
                            ident,
                        )
                    nc.vector.tensor_copy(
                        out=xt[cc][:, tt4 * 512 : (tt4 + 1) * 512], in_=trp
                    )

            # wp DMA early; casts are emitted later (after attention(1)) so
            # the ACT FIFO never blocks pair-0/1 exps on a late DMA
            wp_stages = []
            wp_bf = []
            for g in range(CC):
                s = wstage.tile([128, C], F32, tag="wps", name=f"wps{g}")
                nc.sync.dma_start(out=s, in_=wp_d[g * 128 : (g + 1) * 128, :])
                wp_stages.append(s)

            def emit_wp_casts():
                for g in range(CC):
                    b = wp_pool.tile([128, C], BF16, name=f"wpbf{g}")
                    nc.scalar.copy(out=b, in_=wp_stages[g])
                    wp_bf.append(b)

            ones12 = singles.tile([128, H], BF16)
            nc.gpsimd.memset(ones12, 1.0)

            # ---- qkT: per pair, W-stationary matmuls ----
            qkt = {}

            def emit_qkT(p):
                for which, slicer in [("q", wq_slice), ("k", wk_slice)]:
                    dst = qkt_pool.tile([128, T], BF16, tag=which, name=f"{which}{p}")
                    for tch in range(2):
                        ps = ps_flex.tile([128, 512], F32, tag="flex", name="psqk")
                        for cc in range(CC):
                            nc.tensor.matmul(
                                ps,
                                slicer(p, cc),
                                xt[cc][:, tch * 512 : (tch + 1) * 512],
                                start=(cc == 0),
                                stop=(cc == CC - 1),
                            )
                        nc.vector.tensor_copy(
                            out=dst[:, tch * 512 : (tch + 1) * 512], in_=ps
                        )
                    qkt[(p, which)] = dst

            emit_warmup(24)
            emit_qkT(0)
            emit_warmup(24)
            emit_qkT(1)

            # ---- vhat: x-stationary @ Wv, ones col per head ----
            vhat = [None] * TT

            def emit_vhat(tts):
                for tt in tts:
                    vh = vh_pool.tile([128, H * 65], BF16, name=f"vh{tt}")
                    vhv = vh.rearrange("p (h e) -> p h e", e=65)
                    nc.vector.tensor_copy(
                        out=vhv[:, :, 64:65], in_=ones12.unsqueeze(2)
                    )
                    for n0, nw in [(0, 512), (512, 256)]:
                        ps = ps_flex.tile([128, 512], F32, tag="flex", name="psv")
                        for cc in range(CC):
                            nc.tensor.matmul(
                                ps[:, 0:nw],
                                xt[cc][:, tt * 128 : (tt + 1) * 128],
                                wv_bf[cc][:, n0 : n0 + nw],
                                start=(cc == 0),
                                stop=(cc == CC - 1),
                            )
                        h0 = n0 // 64
                        nh = nw // 64
                        nc.vector.tensor_copy(
                            out=vhv[:, h0 : h0 + nh, 0:64],
                            in_=ps[:, 0:nw].rearrange("p (h e) -> p h e", e=64),
                        )
                    vhat[tt] = vh

            emit_vhat(range(0, 4))

            ypair = []
            for p in range(PAIRS):
                yp = yp_pool.tile([128, T], BF16, name=f"yp{p}")
                ypair.append(yp)

            # ---- attention per pair: row-tiled ST, q-half PV ----
            def emit_attention_qh(p, qh, filler=None):
                qt = qkt[(p, "q")]
                kt = qkt[(p, "k")]
                if True:
                    q0 = 512 * qh
                    i_list = range(4) if qh == 0 else range(8)
                    pvA = ps_pv.tile([65, 512], F32, tag="pv", name="pvA")
                    pvB = ps_pv.tile([65, 512], F32, tag="pv", name="pvB")
                    n_i = len(i_list)

                    def emit_pv(step):
                        i, off, len_, ptA, ptB = step
                        vv = vhat[i].rearrange("p (h e) -> p h e", e=65)
                        for hh, (pv_t, pt_t) in ((0, (pvA, ptA)), (1, (pvB, ptB))):
                            nc.tensor.matmul(
                                pv_t[0:65, off : off + len_],
                                vv[:, 2 * p + hh, :],
                                pt_t[:, 0:len_],
                                start=(i == 0),
                                stop=(i == n_i - 1),
                            )

                    prev = None
                    for i in i_list:
                        if filler is not None:
                            filler()
                        k0 = 128 * i
                        if qh == 0:
                            off = k0
                        else:
                            off = 0 if i <= 3 else k0 - 512
                        len_ = 512 - off
                        diag = (qh == 0) or (i >= 4)
                        stA = ps_st.tile([128, 512], F32, tag="st", name="stA")
                        stB = ps_st.tile([128, 512], F32, tag="st", name="stB")
                        nc.tensor.matmul(
                            stA[:, 0:len_],
                            kt[0:64, k0 : k0 + 128],
                            qt[0:64, q0 + off : q0 + off + len_],
                            start=True,
                            stop=True,
                        )
                        nc.tensor.matmul(
                            stB[:, 0:len_],
                            kt[64:128, k0 : k0 + 128],
                            qt[64:128, q0 + off : q0 + off + len_],
                            start=True,
                            stop=True,
                        )
                        ptA = pt_pool.tile([128, 512], BF16, tag="pt", name="ptA")
                        ptB = pt_pool.tile([128, 512], BF16, tag="pt", name="ptB")
                        nc.scalar.activation(
                            out=ptA[:, 0:len_],
                            in_=stA[:, 0:len_],
                            func=AF.Exp,
                            scale=0.125,
                        )
                        nc.scalar.activation(
                            out=ptB[:, 0:len_],
                            in_=stB[:, 0:len_],
                            func=AF.Exp,
                            scale=0.125,
                        )
                        if diag:
                            for pt_t in (ptA, ptB):
                                nc.gpsimd.affine_select(
                                    out=pt_t[:, 0:128],
                                    in_=pt_t[:, 0:128],
                                    compare_op=mybir.AluOpType.is_ge,
                                    fill=0.0,
                                    base=0,
                                    pattern=[[1, 128]],
                                    channel_multiplier=-1,
                                )
                        if prev is not None:
                            emit_pv(prev)
                        prev = (i, off, len_, ptA, ptB)
                    emit_pv(prev)

                    # finalize this q-half: yT copies, recip, bcast, scale
                    rec = rec_pool.tile([1, 1024], F32, tag="rec", name="rec")
                    nc.vector.tensor_copy(
                        out=ypair[p][0:64, q0 : q0 + 512], in_=pvA[0:64, :]
                    )
                    nc.vector.tensor_copy(
                        out=ypair[p][64:128, q0 : q0 + 512], in_=pvB[0:64, :]
                    )
                    den = rec_pool.tile([1, 1024], F32, tag="den", name="den")
                    nc.vector.tensor_copy(out=den[0:1, 0:512], in_=pvA[64:65, :])
                    nc.vector.tensor_copy(out=den[0:1, 512:1024], in_=pvB[64:65, :])
                    nc.vector.reciprocal_approx_fast(out=rec[0:1, :], in_=den[0:1, :])
                    bc = bc_pool.tile([128, 1024], F32, tag="bc", name="bc")
                    nc.gpsimd.partition_broadcast(bc, rec[0:1, :])
                    nc.vector.tensor_mul(
                        ypair[p][0:64, q0 : q0 + 512],
                        ypair[p][0:64, q0 : q0 + 512],
                        bc[0:64, 0:512],
                    )
                    nc.vector.tensor_mul(
                        ypair[p][64:128, q0 : q0 + 512],
                        ypair[p][64:128, q0 : q0 + 512],
                        bc[64:128, 512:1024],
                    )

            # ---- proj helpers ----
            part = []
            for tt in range(TT):
                pt_ = part_pool.tile([128, C], BF16, name=f"part{tt}")
                part.append(pt_)

            # pairs 0..3 accumulated into SBUF partials, emitted one chunk at
            # a time as filler inside pairs 4/5 (keeps the PE mix dense so
            # HAM stays at K=8/8 through the tail)
            part_chunks = [
                (tt, n0, nw) for tt in range(TT) for n0, nw in [(0, 512), (512, 256)]
            ]
            part_idx = [0]

            def emit_partial_chunk():
                if part_idx[0] >= len(part_chunks):
                    return
                tt, n0, nw = part_chunks[part_idx[0]]
                part_idx[0] += 1
                ps = ps_flex.tile([128, 512], F32, tag="flex", name="pspp")
                for g in range(4):
                    nc.tensor.matmul(
                        ps[:, 0:nw],
                        ypair[g][:, tt * 128 : (tt + 1) * 128],
                        wp_bf[g][:, n0 : n0 + nw],
                        start=(g == 0),
                        stop=(g == 3),
                    )
                nc.vector.tensor_copy(
                    out=part[tt][:, n0 : n0 + nw], in_=ps[:, 0:nw]
                )

            def emit_final_proj(tts):
                for tt in tts:
                    outs = outst_pool.tile([128, C], F32, tag="outs", name="outs")
                    for n0, nw in [(0, 512), (512, 256)]:
                        ps = ps_st.tile([128, 512], F32, tag="st", name="psfp")
                        for g in (4, 5):
                            nc.tensor.matmul(
                                ps[:, 0:nw],
                                ypair[g][:, tt * 128 : (tt + 1) * 128],
                                wp_bf[g][:, n0 : n0 + nw],
                                start=(g == 4),
                                stop=(g == 5),
                            )
                        nc.vector.tensor_add(
                            outs[:, n0 : n0 + nw],
                            ps[:, 0:nw],
                            part[tt][:, n0 : n0 + nw],
                        )
                    nc.sync.dma_start(
                        out=out_d[tt * 128 : (tt + 1) * 128, :], in_=outs
                    )

            # ---- main schedule ----
            emit_attention_qh(0, 0)
            emit_vhat(range(4, 8))
            emit_attention_qh(0, 1)
            emit_qkT(2)
            emit_attention_qh(1, 0)
            emit_attention_qh(1, 1)
            emit_qkT(3)
            emit_wp_casts()
            emit_attention_qh(2, 0)
            emit_attention_qh(2, 1)
            emit_qkT(4)
            emit_attention_qh(3, 0)
            emit_attention_qh(3, 1)
            emit_qkT(5)
            for p, qh in [(4, 0), (4, 1), (5, 0)]:
                emit_attention_qh(p, qh, filler=emit_partial_chunk)
            while part_idx[0] < len(part_chunks):
                emit_partial_chunk()
            emit_attention_qh(5, 1, filler=emit_partial_chunk)
            # t-chunks 0..3 of the output only need ypair[5] cols < 512,
            # scaled right after pair-5 qh0 - they overlap pair-5 qh1
            emit_final_proj(range(0, 4))
            emit_final_proj(range(4, 8))
            # lowest-priority drip warmup: the scheduler slots these into PE
            # idle gaps (mostly the DMA-paced start), keeping HAM at K=8/8
            for _ in range(20):
                emit_warmup(4)

    nc.compile()
    return nc


_NC_CACHE = None


def _get_nc():
    global _NC_CACHE
    if _NC_CACHE is None:
        _NC_CACHE = build_nc()
    return _NC_CACHE


def kernel(**inputs):
    from concourse.bass_utils import run_bass_kernel_spmd

    x = np.asarray(inputs["x"], dtype=np.float32)
    wa = np.ascontiguousarray(np.asarray(inputs["W_attn"], dtype=np.float32))
    wpj = np.ascontiguousarray(np.asarray(inputs["W_proj"], dtype=np.float32))
    B = x.shape[0]
    assert x.shape == (B, T, C) and B == 8

    nc = _get_nc()
    in_maps = [
        {"x": np.ascontiguousarray(x[b]), "wa": wa, "wp": wpj} for b in range(B)
    ]
    res = run_bass_kernel_spmd(nc, in_maps, list(range(B)))
    out = np.stack([res.results[b]["out"] for b in range(B)], axis=0)
    return out.astype(np.float32)
